# revision 11
# baseline (speedup 1.0000x reference)
"""Trainium2 Bass kernel for nn_MultiHeadSelfTokenAttention.

Reference computation (per (b, s) slice, X = hidden[b, s] in [T=128, H=768]):
    q      = X @ Wq + bq                       [T, 12]     (per-token per-head logit)
    scores = q + mask[:, None] * (-10000)
    alpha  = softmax(scores, axis=T)           [T, 12]
    v      = (X @ Wv + bv).reshape(T, 12, 64)
    res    = einsum('th,thd->hd', alpha, v)    [12, 64] -> [768]
    out    = LN(res @ Wo + bo) * gamma + beta  [768]

Key algebraic restructure (makes the kernel memory-bound instead of
compute-bound): the pooled value P = sum_t alpha * V is computed as
    Y[head, h] = sum_t alpha[t, head] * X[t, h]
    P[head, :] = Y[head, :] @ Wv[:, head*64:(head+1)*64] + bv_head
so V ([T, 768] per slice) is never materialized; the X@Wv matmul
(151 MFLOP/slice) collapses to ~2.4 MFLOP/slice.

Sharding: data-parallel across batch; core b handles hidden_states[b]
(32 sents).  Weights replicated.  No collectives.
"""

import os
import sys
from contextlib import ExitStack

import numpy as np

for _p in ("/opt/trn_rl_repo", "/root/.axon_site/_ro/trn_rl_repo"):
    if os.path.isdir(_p) and _p not in sys.path:
        sys.path.insert(0, _p)

import concourse.bacc as bacc
import concourse.bass as bass
import concourse.tile as tile
from concourse import mybir
from concourse.bass_utils import run_bass_kernel_spmd

F32 = mybir.dt.float32
F32R = mybir.dt.float32r
AF = mybir.ActivationFunctionType
ALU = mybir.AluOpType

HIDDEN = 768
HEADS = 12
B, S, T = 8, 32, 128
HC = HIDDEN // 128  # 6 chunks of the hidden dim
LN_EPS = 1e-5
MASK_NEG = -10000.0
N_CORES = 8
BS = 4  # sents per block
NBLK = S // BS

# dtype knobs for PE matmuls (float32 = exact 2-pass, float32r = fast 1-pass)
MM_DT = F32  # q^T / Y / G / out-projection matmuls
TR_DT = F32  # PE transposes


def _c(ap, dt):
    return ap.bitcast(dt) if dt != F32 else ap


def build_kernel():
    nc = bacc.Bacc(trn_type="TRN2", target_bir_lowering=False, debug=False)

    hs = nc.dram_tensor("hs", [S, T, HIDDEN], F32, kind="ExternalInput").ap()
    mask = nc.dram_tensor("mask", [S, T], F32, kind="ExternalInput").ap()
    wq = nc.dram_tensor("wq", [HIDDEN, HEADS], F32, kind="ExternalInput").ap()
    bq = nc.dram_tensor("bq", [HEADS], F32, kind="ExternalInput").ap()
    wv = nc.dram_tensor("wv", [HIDDEN, HIDDEN], F32, kind="ExternalInput").ap()
    bv = nc.dram_tensor("bv", [HIDDEN], F32, kind="ExternalInput").ap()
    wo = nc.dram_tensor("wo", [HIDDEN, HIDDEN], F32, kind="ExternalInput").ap()
    bo = nc.dram_tensor("bo", [HIDDEN], F32, kind="ExternalInput").ap()
    gamma = nc.dram_tensor("gamma", [HIDDEN], F32, kind="ExternalInput").ap()
    beta = nc.dram_tensor("beta", [HIDDEN], F32, kind="ExternalInput").ap()
    ident = nc.dram_tensor("ident", [128, 128], F32, kind="ExternalInput").ap()
    out = nc.dram_tensor("out", [S, HIDDEN], F32, kind="ExternalOutput").ap()

    with tile.TileContext(nc) as tc:
        kernel_body(tc, out, hs, mask, wq, bq, wv, bv, wo, bo, gamma, beta, ident)
    nc.compile()
    return nc


def kernel_body(tc, out, hs, mask, wq, bq, wv, bv, wo, bo, gamma, beta, ident):
    nc = tc.nc
    with ExitStack() as ctx:
        consts = ctx.enter_context(tc.tile_pool(name="consts", bufs=1))
        xp = ctx.enter_context(tc.tile_pool(name="x", bufs=2))
        xtp = ctx.enter_context(tc.tile_pool(name="xt", bufs=2))
        smallp = ctx.enter_context(tc.tile_pool(name="small", bufs=2))
        psctx = ExitStack()
        ps_xt = psctx.enter_context(tc.tile_pool(name="ps_xt", bufs=1, space="PSUM"))
        ps_qt = psctx.enter_context(tc.tile_pool(name="ps_qt", bufs=1, space="PSUM"))
        ps_et = psctx.enter_context(tc.tile_pool(name="ps_et", bufs=1, space="PSUM"))
        ps_y = psctx.enter_context(tc.tile_pool(name="ps_y", bufs=1, space="PSUM"))
        ps_yt = psctx.enter_context(tc.tile_pool(name="ps_yt", bufs=1, space="PSUM"))

        # ---------------- constants / weights -------------------------------
        ident_sb = consts.tile([128, 128], F32, tag="ident")
        nc.sync.dma_start(ident_sb[:], ident[:])

        # wq_sb[p, c*12+n] = Wq[c*128+p, n]
        wq_sb = consts.tile([128, HC * HEADS], F32, tag="wq")
        nc.sync.dma_start(wq_sb[:], wq.rearrange("(c p) n -> p c n", p=128))

        # wv_sb[p, c*768+d] = Wv[c*128+p, d]
        wv_sb = consts.tile([128, HC * HIDDEN], F32, tag="wv")
        nc.sync.dma_start(wv_sb[:], wv.rearrange("(c p) n -> p c n", p=128))

        wo_sb = consts.tile([128, HC * HIDDEN], F32, tag="wo")
        nc.sync.dma_start(wo_sb[:], wo.rearrange("(c p) n -> p c n", p=128))

        # extras matmul operands: scores^T += [NEG; bq]^T-style rank-2 update
        extras_w = consts.tile([2, HEADS], F32, tag="exw")
        nc.vector.memset(extras_w[0:1, :], MASK_NEG)
        nc.sync.dma_start(extras_w[1:2, :], bq[None, :])
        extras_rhs = consts.tile([2, S * T], F32, tag="exr")
        nc.vector.memset(extras_rhs[:], 1.0)  # row 1 stays all-ones
        nc.sync.dma_start(extras_rhs[0:1, :], mask.rearrange("s t -> (s t)")[None, :])

        ones_col = consts.tile([1, S], F32, tag="ones")
        nc.vector.memset(ones_col[:], 1.0)
        bo_row = consts.tile([1, HIDDEN], F32, tag="bo")
        nc.sync.dma_start(bo_row[:], bo[None, :])
        g_row = consts.tile([1, HIDDEN], F32, tag="grow")
        nc.sync.dma_start(g_row[:], gamma[None, :])
        b_row = consts.tile([1, HIDDEN], F32, tag="brow")
        nc.sync.dma_start(b_row[:], beta[None, :])

        # bv_sb[p, c] = bv[c*128+p]
        bv_sb = consts.tile([128, HC], F32, tag="bv")
        nc.sync.dma_start(bv_sb[:], bv.rearrange("(c p) -> p c", p=128))

        # gamma/beta replicated across the 32 sent-partitions via a K=1 matmul
        gamma_rep = consts.tile([S, HIDDEN], F32, tag="grep")
        beta_rep = consts.tile([S, HIDDEN], F32, tag="brep")
        for row, rep in ((g_row, gamma_rep), (b_row, beta_rep)):
            gb1 = ps_y.tile([S, 512], F32, tag="y1", name="gb1")
            gb2 = ps_y.tile([S, 256], F32, tag="y2", name="gb2")
            nc.tensor.matmul(gb1[:], ones_col[:], row[:, 0:512])
            nc.tensor.matmul(gb2[:], ones_col[:], row[:, 512:768])
            nc.vector.tensor_copy(rep[:, 0:512], gb1[:])
            nc.scalar.copy(rep[:, 512:768], gb2[:])

        # Y^T accumulator in SBUF: per h-chunk [128, S*HEADS], col = s*12 + head
        yt_sb = [
            consts.tile([128, S * HEADS], F32, tag=f"yt{c}", name=f"yt{c}")
            for c in range(HC)
        ]

        # ---------------- main loop over sent blocks ------------------------
        for blk in range(NBLK):
            s0 = blk * BS
            # X block: [t, (s', h)]  (contiguous DMA, 1.57 MB)
            x_blk = xp.tile([128, BS * HIDDEN], F32, tag="xblk", name="x_blk")
            nc.sync.dma_start(x_blk[:], hs[s0 : s0 + BS].rearrange("s t h -> t s h"))

            # X^T block in SBUF: col = s'*768 + hc*128 + j
            xt_blk = xtp.tile([128, BS * HIDDEN], F32, tag="xtblk", name="xt_blk")
            for sp in range(BS):
                xt_a = ps_xt.tile([128, 512], F32, tag="xt_a", name="xt_a")
                xt_b = ps_xt.tile([128, 256], F32, tag="xt_b", name="xt_b")
                for c in range(HC):
                    dst = (
                        xt_a[:, c * 128 : (c + 1) * 128]
                        if c < 4
                        else xt_b[:, (c - 4) * 128 : (c - 3) * 128]
                    )
                    nc.tensor.transpose(
                        dst,
                        _c(
                            x_blk[
                                :, sp * HIDDEN + c * 128 : sp * HIDDEN + (c + 1) * 128
                            ],
                            TR_DT,
                        ),
                        _c(ident_sb[:], TR_DT),
                    )
                nc.vector.tensor_copy(
                    xt_blk[:, sp * HIDDEN : sp * HIDDEN + 512], xt_a[:]
                )
                nc.scalar.copy(
                    xt_blk[:, sp * HIDDEN + 512 : (sp + 1) * HIDDEN], xt_b[:]
                )

            # q^T for the whole block: [12, BS*128]
            qt_ps = ps_qt.tile([HEADS, BS * T], F32, tag="qt", name="qt_ps")
            xt_r = xt_blk.rearrange("p (s c j) -> p c s j", s=BS, j=128)
            for c in range(HC):
                nc.tensor.matmul(
                    qt_ps[:],
                    _c(wq_sb[:, c * HEADS : (c + 1) * HEADS], MM_DT),
                    _c(xt_r[:, c], MM_DT),
                    start=(c == 0),
                    stop=False,
                )
            nc.tensor.matmul(
                qt_ps[:],
                _c(extras_w[:], MM_DT),
                _c(extras_rhs[:, s0 * T : (s0 + BS) * T], MM_DT),
                start=False,
                stop=True,
            )

            # softmax pieces (no max-subtraction: unmasked logits are O(5);
            # masked logits are ~-1e4 and exp underflows to exactly 0,
            # matching the reference's max-subtracted exp)
            et_sb = smallp.tile([HEADS, BS * T], F32, tag="et", name="et_sb")
            zsum = smallp.tile([HEADS, BS], F32, tag="zsum", name="zsum")
            for sp in range(BS):
                nc.scalar.activation(
                    et_sb[:, sp * T : (sp + 1) * T],
                    qt_ps[:, sp * T : (sp + 1) * T],
                    AF.Exp,
                    accum_out=zsum[:, sp : sp + 1],
                )
            zinv = smallp.tile([HEADS, BS], F32, tag="zinv", name="zinv")
            nc.vector.reciprocal(zinv[:], zsum[:])

            # e^T transposed back to [t, head] layout for the Y matmul
            ett_ps = ps_et.tile([128, BS * HEADS], F32, tag="ett", name="ett_ps")
            for sp in range(BS):
                nc.tensor.transpose(
                    ett_ps[:, sp * HEADS : (sp + 1) * HEADS],
                    _c(et_sb[:, sp * T : (sp + 1) * T], TR_DT),
                    _c(ident_sb[0:HEADS, 0:HEADS], TR_DT),
                )
            e_sb = smallp.tile([128, BS * HEADS], F32, tag="e", name="e_sb")
            nc.vector.tensor_copy(e_sb[:], ett_ps[:])

            yt_a = ps_yt.tile([128, 3 * BS * HEADS], F32, tag="yt_a", name="yt_a")
            yt_b = ps_yt.tile([128, 3 * BS * HEADS], F32, tag="yt_b", name="yt_b")
            for sp in range(BS):
                # Y = e^T X  -> [12, 768] (normalization by 1/Z folded in below)
                y1 = ps_y.tile([HEADS, 512], F32, tag="y1", name="y1")
                y2 = ps_y.tile([HEADS, 256], F32, tag="y2", name="y2")
                nc.tensor.matmul(
                    y1[:],
                    _c(e_sb[:, sp * HEADS : (sp + 1) * HEADS], MM_DT),
                    _c(x_blk[:, sp * HIDDEN : sp * HIDDEN + 512], MM_DT),
                )
                nc.tensor.matmul(
                    y2[:],
                    _c(e_sb[:, sp * HEADS : (sp + 1) * HEADS], MM_DT),
                    _c(x_blk[:, sp * HIDDEN + 512 : (sp + 1) * HIDDEN], MM_DT),
                )
                y_sb = smallp.tile([HEADS, HIDDEN], F32, tag="ysb", name="y_sb")
                nc.vector.tensor_scalar_mul(
                    y_sb[:, 0:512], y1[:], zinv[:, sp : sp + 1]
                )
                nc.scalar.activation(
                    y_sb[:, 512:768], y2[:], AF.Copy, scale=zinv[:, sp : sp + 1]
                )
                # transpose Y into the Y^T psum accumulators
                for c in range(HC):
                    bank = yt_a if c < 3 else yt_b
                    cc = c % 3
                    nc.tensor.transpose(
                        bank[
                            :,
                            cc * BS * HEADS
                            + sp * HEADS : cc * BS * HEADS
                            + (sp + 1) * HEADS,
                        ],
                        _c(y_sb[:, c * 128 : (c + 1) * 128], TR_DT),
                        _c(ident_sb[0:HEADS, 0:HEADS], TR_DT),
                    )
            for c in range(HC):
                bank = yt_a if c < 3 else yt_b
                cc = c % 3
                eng = nc.vector.tensor_copy if c % 2 == 0 else nc.scalar.copy
                eng(
                    yt_sb[c][:, s0 * HEADS : (s0 + BS) * HEADS],
                    bank[:, cc * BS * HEADS : (cc + 1) * BS * HEADS],
                )

        psctx.close()  # free the main-loop PSUM banks before stage G

        # ---------------- pooled projection through Wv (G-route) ------------
        # G^T[d, (s,head)] = sum_h Wv[h, d] * Y^T[h, (s,head)]; per-head
        # diagonal blocks extracted:  P^T[d, s] = G^T[d, s*12 + head(d)] + bv[d]
        with (
            tc.tile_pool(name="ps_g", bufs=2, space="PSUM") as ps_g,
            tc.tile_pool(name="ps_o", bufs=1, space="PSUM") as ps_o,
            tc.tile_pool(name="fin", bufs=1) as fin,
        ):
            pt_sb = fin.tile([128, HC * S], F32, tag="pt", name="pt_sb")
            for dc in range(HC):
                g_ps = ps_g.tile([128, S * HEADS], F32, tag="g", name="g_ps")
                for c in range(HC):
                    nc.tensor.matmul(
                        g_ps[:],
                        _c(
                            wv_sb[
                                :,
                                c * HIDDEN + dc * 128 : c * HIDDEN + (dc + 1) * 128,
                            ],
                            MM_DT,
                        ),
                        _c(yt_sb[c][:], MM_DT),
                        start=(c == 0),
                        stop=(c == HC - 1),
                    )
                g_r = g_ps.rearrange("p (s n) -> p s n", n=HEADS)
                for half in range(2):
                    head = 2 * dc + half
                    nc.vector.tensor_scalar_add(
                        pt_sb[half * 64 : half * 64 + 64, dc * S : (dc + 1) * S],
                        g_r[half * 64 : half * 64 + 64, :, head],
                        bv_sb[half * 64 : half * 64 + 64, dc : dc + 1],
                    )

            # out = P @ Wo + bo   -> [32, 768]
            o1 = ps_o.tile([S, 512], F32, tag="o1", name="o1")
            o2 = ps_o.tile([S, 256], F32, tag="o2", name="o2")
            for dc in range(HC):
                nc.tensor.matmul(
                    o1[:],
                    _c(pt_sb[:, dc * S : (dc + 1) * S], MM_DT),
                    _c(wo_sb[:, dc * HIDDEN : dc * HIDDEN + 512], MM_DT),
                    start=(dc == 0),
                    stop=False,
                )
                nc.tensor.matmul(
                    o2[:],
                    _c(pt_sb[:, dc * S : (dc + 1) * S], MM_DT),
                    _c(wo_sb[:, dc * HIDDEN + 512 : (dc + 1) * HIDDEN], MM_DT),
                    start=(dc == 0),
                    stop=False,
                )
            nc.tensor.matmul(o1[:], ones_col[:], bo_row[:, 0:512], start=False, stop=True)
            nc.tensor.matmul(
                o2[:], ones_col[:], bo_row[:, 512:768], start=False, stop=True
            )

            # ---------------- layernorm ------------------------------------
            res_sb = fin.tile([S, HIDDEN], F32, tag="res", name="res_sb")
            mu_parts = fin.tile([S, 2], F32, tag="mup", name="mu_parts")
            nc.scalar.activation(
                res_sb[:, 0:512], o1[:], AF.Copy, accum_out=mu_parts[:, 0:1]
            )
            nc.scalar.activation(
                res_sb[:, 512:768], o2[:], AF.Copy, accum_out=mu_parts[:, 1:2]
            )
            mu = fin.tile([S, 1], F32, tag="mu", name="mu")
            nc.vector.tensor_reduce(
                mu[:], mu_parts[:], axis=mybir.AxisListType.X, op=ALU.add
            )
            muv = fin.tile([S, 1], F32, tag="muv", name="muv")
            nc.vector.tensor_scalar_mul(muv[:], mu[:], 1.0 / HIDDEN)
            xc = fin.tile([S, HIDDEN], F32, tag="xc", name="xc")
            nc.vector.tensor_scalar_sub(xc[:], res_sb[:], muv[:])
            sq = fin.tile([S, HIDDEN], F32, tag="sq", name="sq")
            varsum = fin.tile([S, 1], F32, tag="vs", name="varsum")
            nc.scalar.activation(sq[:], xc[:], AF.Square, accum_out=varsum[:])
            vareps = fin.tile([S, 1], F32, tag="ve", name="vareps")
            nc.vector.tensor_scalar(
                vareps[:], varsum[:], 1.0 / HIDDEN, LN_EPS, op0=ALU.mult, op1=ALU.add
            )
            sd = fin.tile([S, 1], F32, tag="sd", name="sd")
            nc.scalar.activation(sd[:], vareps[:], AF.Sqrt)
            rstd = fin.tile([S, 1], F32, tag="rstd", name="rstd")
            nc.vector.reciprocal(rstd[:], sd[:])
            t1 = fin.tile([S, HIDDEN], F32, tag="t1", name="t1")
            nc.vector.scalar_tensor_tensor(
                t1[:], xc[:], rstd[:], gamma_rep[:], op0=ALU.mult, op1=ALU.mult
            )
            out_sb = fin.tile([S, HIDDEN], F32, tag="osb", name="out_sb")
            nc.vector.tensor_add(out_sb[:], t1[:], beta_rep[:])
            nc.sync.dma_start(out[:], out_sb[:])


_NC_CACHE = {}


def kernel(hidden_states, mask, Wq, bq, Wv, bv, Wo, bo, gamma, beta):
    if "nc" not in _NC_CACHE:
        _NC_CACHE["nc"] = build_kernel()
    nc = _NC_CACHE["nc"]
    ident = np.eye(128, dtype=np.float32)
    f32 = np.float32

    def cc(a):
        return np.ascontiguousarray(a, dtype=f32)

    in_maps = [
        {
            "hs": cc(hidden_states[b]),
            "mask": cc(mask[b]),
            "wq": cc(Wq),
            "bq": cc(bq),
            "wv": cc(Wv),
            "bv": cc(bv),
            "wo": cc(Wo),
            "bo": cc(bo),
            "gamma": cc(gamma),
            "beta": cc(beta),
            "ident": ident,
        }
        for b in range(N_CORES)
    ]
    res = run_bass_kernel_spmd(nc, in_maps, core_ids=list(range(N_CORES)))
    _NC_CACHE["last_results"] = res
    globals()["_LAST_RESULTS"] = res
    return np.stack([res.results[i]["out"] for i in range(N_CORES)], axis=0)


# revision 14
# speedup vs baseline: 1.2176x; 1.2176x over previous
"""Trainium2 Bass kernel for nn_MultiHeadSelfTokenAttention.

Reference computation (per (b, s) slice, X = hidden[b, s] in [T=128, H=768]):
    q      = X @ Wq + bq                       [T, 12]     (per-token per-head logit)
    scores = q + mask[:, None] * (-10000)
    alpha  = softmax(scores, axis=T)           [T, 12]
    v      = (X @ Wv + bv).reshape(T, 12, 64)
    res    = einsum('th,thd->hd', alpha, v)    [12, 64] -> [768]
    out    = LN(res @ Wo + bo) * gamma + beta  [768]

Key algebraic restructure (makes the kernel memory-bound instead of
compute-bound): the pooled value P = sum_t alpha * V is computed as
    Y[head, h] = sum_t alpha[t, head] * X[t, h]
    P[head, :] = Y[head, :] @ Wv[:, head*64:(head+1)*64] + bv_head
so V ([T, 768] per slice) is never materialized; the X@Wv matmul
(151 MFLOP/slice) collapses to ~2.4 MFLOP/slice.

Sharding: data-parallel across batch; core b handles hidden_states[b]
(32 sents).  Weights replicated.  No collectives.
"""

import os
import sys
from contextlib import ExitStack

import numpy as np

for _p in ("/opt/trn_rl_repo", "/root/.axon_site/_ro/trn_rl_repo"):
    if os.path.isdir(_p) and _p not in sys.path:
        sys.path.insert(0, _p)

import concourse.bacc as bacc
import concourse.bass as bass
import concourse.tile as tile
from concourse import mybir
from concourse.bass_utils import run_bass_kernel_spmd

F32 = mybir.dt.float32
F32R = mybir.dt.float32r
AF = mybir.ActivationFunctionType
ALU = mybir.AluOpType

HIDDEN = 768
HEADS = 12
B, S, T = 8, 32, 128
HC = HIDDEN // 128  # 6 chunks of the hidden dim
LN_EPS = 1e-5
MASK_NEG = -10000.0
N_CORES = 8
BS = 4  # sents per block
NBLK = S // BS

# dtype knob: float32r tiles feed 1-pass (4x faster, ~1.5e-4 rel) PE matmuls;
# float32 gives the exact 2-pass path.
MMD = F32R if os.environ.get("KMM", "f32r") == "f32r" else F32


def _f32(ap):
    # exact-bits view of an MMD tile for non-matmul consumers (PE transpose)
    return ap.bitcast(F32) if MMD != F32 else ap


def build_kernel():
    nc = bacc.Bacc(trn_type="TRN2", target_bir_lowering=False, debug=False)

    hs = nc.dram_tensor("hs", [S, T, HIDDEN], MMD, kind="ExternalInput").ap()
    mask = nc.dram_tensor("mask", [S, T], F32, kind="ExternalInput").ap()
    wq = nc.dram_tensor("wq", [HIDDEN, HEADS], MMD, kind="ExternalInput").ap()
    bq = nc.dram_tensor("bq", [HEADS], F32, kind="ExternalInput").ap()
    wv = nc.dram_tensor("wv", [HIDDEN, HIDDEN], MMD, kind="ExternalInput").ap()
    bv = nc.dram_tensor("bv", [HIDDEN], F32, kind="ExternalInput").ap()
    wo = nc.dram_tensor("wo", [HIDDEN, HIDDEN], MMD, kind="ExternalInput").ap()
    bo = nc.dram_tensor("bo", [HIDDEN], F32, kind="ExternalInput").ap()
    gamma = nc.dram_tensor("gamma", [HIDDEN], F32, kind="ExternalInput").ap()
    beta = nc.dram_tensor("beta", [HIDDEN], F32, kind="ExternalInput").ap()
    ident = nc.dram_tensor("ident", [128, 128], F32, kind="ExternalInput").ap()
    out = nc.dram_tensor("out", [S, HIDDEN], F32, kind="ExternalOutput").ap()

    with tile.TileContext(nc) as tc:
        kernel_body(tc, out, hs, mask, wq, bq, wv, bv, wo, bo, gamma, beta, ident)
    nc.compile()
    return nc


def kernel_body(tc, out, hs, mask, wq, bq, wv, bv, wo, bo, gamma, beta, ident):
    nc = tc.nc
    with ExitStack() as ctx:
        consts = ctx.enter_context(tc.tile_pool(name="consts", bufs=1))
        xp = ctx.enter_context(tc.tile_pool(name="x", bufs=2))
        xtp = ctx.enter_context(tc.tile_pool(name="xt", bufs=2))
        smallp = ctx.enter_context(tc.tile_pool(name="small", bufs=2))
        psctx = ExitStack()
        ps_xt = psctx.enter_context(tc.tile_pool(name="ps_xt", bufs=1, space="PSUM"))
        ps_qt = psctx.enter_context(tc.tile_pool(name="ps_qt", bufs=1, space="PSUM"))
        ps_et = psctx.enter_context(tc.tile_pool(name="ps_et", bufs=1, space="PSUM"))
        ps_y = psctx.enter_context(tc.tile_pool(name="ps_y", bufs=1, space="PSUM"))
        ps_yt = psctx.enter_context(tc.tile_pool(name="ps_yt", bufs=1, space="PSUM"))

        # ---------------- constants / weights -------------------------------
        ident_sb = consts.tile([128, 128], F32, tag="ident")
        nc.sync.dma_start(ident_sb[:], ident[:])

        # wq_sb[p, c*12+n] = Wq[c*128+p, n]
        wq_sb = consts.tile([128, HC * HEADS], MMD, tag="wq")
        nc.sync.dma_start(wq_sb[:], wq.rearrange("(c p) n -> p c n", p=128))

        # wv_sb[p, c*768+d] = Wv[c*128+p, d]
        wv_sb = consts.tile([128, HC * HIDDEN], MMD, tag="wv")
        nc.sync.dma_start(wv_sb[:], wv.rearrange("(c p) n -> p c n", p=128))

        wo_sb = consts.tile([128, HC * HIDDEN], MMD, tag="wo")
        nc.sync.dma_start(wo_sb[:], wo.rearrange("(c p) n -> p c n", p=128))

        # extras matmul operands: scores^T += [NEG; bq]^T-style rank-2 update
        extras_w = consts.tile([2, HEADS], F32, tag="exw")
        nc.vector.memset(extras_w[0:1, :], MASK_NEG)
        nc.sync.dma_start(extras_w[1:2, :], bq[None, :])
        extras_rhs = consts.tile([2, S * T], F32, tag="exr")
        nc.vector.memset(extras_rhs[:], 1.0)  # row 1 stays all-ones
        nc.sync.dma_start(extras_rhs[0:1, :], mask.rearrange("s t -> (s t)")[None, :])

        ones_col = consts.tile([1, S], F32, tag="ones")
        nc.vector.memset(ones_col[:], 1.0)
        bo_row = consts.tile([1, HIDDEN], F32, tag="bo")
        nc.sync.dma_start(bo_row[:], bo[None, :])
        g_row = consts.tile([1, HIDDEN], F32, tag="grow")
        nc.sync.dma_start(g_row[:], gamma[None, :])
        b_row = consts.tile([1, HIDDEN], F32, tag="brow")
        nc.sync.dma_start(b_row[:], beta[None, :])

        # bv_sb[p, c] = bv[c*128+p]
        bv_sb = consts.tile([128, HC], F32, tag="bv")
        nc.sync.dma_start(bv_sb[:], bv.rearrange("(c p) -> p c", p=128))

        # gamma/beta replicated across the 32 sent-partitions via a K=1 matmul
        gamma_rep = consts.tile([S, HIDDEN], F32, tag="grep")
        beta_rep = consts.tile([S, HIDDEN], F32, tag="brep")
        for row, rep in ((g_row, gamma_rep), (b_row, beta_rep)):
            gb1 = ps_y.tile([S, 512], F32, tag="y1", name="gb1")
            gb2 = ps_y.tile([S, 256], F32, tag="y2", name="gb2")
            nc.tensor.matmul(gb1[:], ones_col[:], row[:, 0:512])
            nc.tensor.matmul(gb2[:], ones_col[:], row[:, 512:768])
            nc.vector.tensor_copy(rep[:, 0:512], gb1[:])
            nc.scalar.copy(rep[:, 512:768], gb2[:])

        # Y^T accumulator in SBUF: per h-chunk [128, S*HEADS], col = s*12 + head
        yt_sb = [
            consts.tile([128, S * HEADS], MMD, tag=f"yt{c}", name=f"yt{c}")
            for c in range(HC)
        ]

        # ---------------- main loop over sent blocks ------------------------
        for blk in range(NBLK):
            s0 = blk * BS
            # X block: [t, (s', h)]  (contiguous DMA, 1.57 MB)
            x_blk = xp.tile([128, BS * HIDDEN], MMD, tag="xblk", name="x_blk")
            nc.sync.dma_start(x_blk[:], hs[s0 : s0 + BS].rearrange("s t h -> t s h"))

            # X^T block in SBUF: col = s'*768 + hc*128 + j
            xt_blk = xtp.tile([128, BS * HIDDEN], MMD, tag="xtblk", name="xt_blk")
            for sp in range(BS):
                xt_a = ps_xt.tile([128, 512], F32, tag="xt_a", name="xt_a")
                xt_b = ps_xt.tile([128, 256], F32, tag="xt_b", name="xt_b")
                for c in range(HC):
                    dst = (
                        xt_a[:, c * 128 : (c + 1) * 128]
                        if c < 4
                        else xt_b[:, (c - 4) * 128 : (c - 3) * 128]
                    )
                    nc.tensor.transpose(
                        dst,
                        _f32(
                            x_blk[
                                :, sp * HIDDEN + c * 128 : sp * HIDDEN + (c + 1) * 128
                            ]
                        ),
                        ident_sb[:],
                    )
                nc.vector.tensor_copy(
                    xt_blk[:, sp * HIDDEN : sp * HIDDEN + 512], xt_a[:]
                )
                nc.scalar.copy(
                    xt_blk[:, sp * HIDDEN + 512 : (sp + 1) * HIDDEN], xt_b[:]
                )

            # q^T for the whole block: [12, BS*128]
            qt_ps = ps_qt.tile([HEADS, BS * T], F32, tag="qt", name="qt_ps")
            xt_r = xt_blk.rearrange("p (s c j) -> p c s j", s=BS, j=128)
            for c in range(HC):
                nc.tensor.matmul(
                    qt_ps[:],
                    wq_sb[:, c * HEADS : (c + 1) * HEADS],
                    xt_r[:, c],
                    start=(c == 0),
                    stop=False,
                )
            nc.tensor.matmul(
                qt_ps[:],
                extras_w[:],
                extras_rhs[:, s0 * T : (s0 + BS) * T],
                start=False,
                stop=True,
            )

            # softmax pieces (no max-subtraction: unmasked logits are O(5);
            # masked logits are ~-1e4 and exp underflows to exactly 0,
            # matching the reference's max-subtracted exp)
            et_sb = smallp.tile([HEADS, BS * T], F32, tag="et", name="et_sb")
            zsum = smallp.tile([HEADS, BS], F32, tag="zsum", name="zsum")
            for sp in range(BS):
                nc.scalar.activation(
                    et_sb[:, sp * T : (sp + 1) * T],
                    qt_ps[:, sp * T : (sp + 1) * T],
                    AF.Exp,
                    accum_out=zsum[:, sp : sp + 1],
                )
            zinv = smallp.tile([HEADS, BS], F32, tag="zinv", name="zinv")
            nc.vector.reciprocal(zinv[:], zsum[:])

            # e^T transposed back to [t, head] layout for the Y matmul
            ett_ps = ps_et.tile([128, BS * HEADS], F32, tag="ett", name="ett_ps")
            for sp in range(BS):
                nc.tensor.transpose(
                    ett_ps[:, sp * HEADS : (sp + 1) * HEADS],
                    et_sb[:, sp * T : (sp + 1) * T],
                    ident_sb[0:HEADS, 0:HEADS],
                )
            e_sb = smallp.tile([128, BS * HEADS], MMD, tag="e", name="e_sb")
            nc.vector.tensor_copy(e_sb[:], ett_ps[:])

            yt_a = ps_yt.tile([128, 3 * BS * HEADS], F32, tag="yt_a", name="yt_a")
            yt_b = ps_yt.tile([128, 3 * BS * HEADS], F32, tag="yt_b", name="yt_b")
            for sp in range(BS):
                # Y = e^T X  -> [12, 768] (normalization by 1/Z folded in below)
                y1 = ps_y.tile([HEADS, 512], F32, tag="y1", name="y1")
                y2 = ps_y.tile([HEADS, 256], F32, tag="y2", name="y2")
                nc.tensor.matmul(
                    y1[:],
                    e_sb[:, sp * HEADS : (sp + 1) * HEADS],
                    x_blk[:, sp * HIDDEN : sp * HIDDEN + 512],
                )
                nc.tensor.matmul(
                    y2[:],
                    e_sb[:, sp * HEADS : (sp + 1) * HEADS],
                    x_blk[:, sp * HIDDEN + 512 : (sp + 1) * HIDDEN],
                )
                y_sb = smallp.tile([HEADS, HIDDEN], F32, tag="ysb", name="y_sb")
                nc.vector.tensor_scalar_mul(
                    y_sb[:, 0:512], y1[:], zinv[:, sp : sp + 1]
                )
                nc.scalar.activation(
                    y_sb[:, 512:768], y2[:], AF.Copy, scale=zinv[:, sp : sp + 1]
                )
                # transpose Y into the Y^T psum accumulators
                for c in range(HC):
                    bank = yt_a if c < 3 else yt_b
                    cc = c % 3
                    nc.tensor.transpose(
                        bank[
                            :,
                            cc * BS * HEADS
                            + sp * HEADS : cc * BS * HEADS
                            + (sp + 1) * HEADS,
                        ],
                        y_sb[:, c * 128 : (c + 1) * 128],
                        ident_sb[0:HEADS, 0:HEADS],
                    )
            for c in range(HC):
                bank = yt_a if c < 3 else yt_b
                cc = c % 3
                eng = nc.vector.tensor_copy if c % 2 == 0 else nc.scalar.copy
                eng(
                    yt_sb[c][:, s0 * HEADS : (s0 + BS) * HEADS],
                    bank[:, cc * BS * HEADS : (cc + 1) * BS * HEADS],
                )

        psctx.close()  # free the main-loop PSUM banks before stage G

        # ---------------- pooled projection through Wv (G-route) ------------
        # G^T[d, (s,head)] = sum_h Wv[h, d] * Y^T[h, (s,head)]; per-head
        # diagonal blocks extracted:  P^T[d, s] = G^T[d, s*12 + head(d)] + bv[d]
        with (
            tc.tile_pool(name="ps_g", bufs=2, space="PSUM") as ps_g,
            tc.tile_pool(name="ps_o", bufs=1, space="PSUM") as ps_o,
            tc.tile_pool(name="fin", bufs=1) as fin,
        ):
            pt_sb = fin.tile([128, HC * S], MMD, tag="pt", name="pt_sb")
            for dc in range(HC):
                g_ps = ps_g.tile([128, S * HEADS], F32, tag="g", name="g_ps")
                for c in range(HC):
                    nc.tensor.matmul(
                        g_ps[:],
                        wv_sb[
                            :, c * HIDDEN + dc * 128 : c * HIDDEN + (dc + 1) * 128
                        ],
                        yt_sb[c][:],
                        start=(c == 0),
                        stop=(c == HC - 1),
                    )
                g_r = g_ps.rearrange("p (s n) -> p s n", n=HEADS)
                for half in range(2):
                    head = 2 * dc + half
                    nc.vector.tensor_scalar_add(
                        pt_sb[half * 64 : half * 64 + 64, dc * S : (dc + 1) * S],
                        g_r[half * 64 : half * 64 + 64, :, head],
                        bv_sb[half * 64 : half * 64 + 64, dc : dc + 1],
                    )

            # out = P @ Wo + bo   -> [32, 768]
            o1 = ps_o.tile([S, 512], F32, tag="o1", name="o1")
            o2 = ps_o.tile([S, 256], F32, tag="o2", name="o2")
            for dc in range(HC):
                nc.tensor.matmul(
                    o1[:],
                    pt_sb[:, dc * S : (dc + 1) * S],
                    wo_sb[:, dc * HIDDEN : dc * HIDDEN + 512],
                    start=(dc == 0),
                    stop=False,
                )
                nc.tensor.matmul(
                    o2[:],
                    pt_sb[:, dc * S : (dc + 1) * S],
                    wo_sb[:, dc * HIDDEN + 512 : (dc + 1) * HIDDEN],
                    start=(dc == 0),
                    stop=False,
                )
            nc.tensor.matmul(o1[:], ones_col[:], bo_row[:, 0:512], start=False, stop=True)
            nc.tensor.matmul(
                o2[:], ones_col[:], bo_row[:, 512:768], start=False, stop=True
            )

            # ---------------- layernorm ------------------------------------
            res_sb = fin.tile([S, HIDDEN], F32, tag="res", name="res_sb")
            mu_parts = fin.tile([S, 2], F32, tag="mup", name="mu_parts")
            nc.scalar.activation(
                res_sb[:, 0:512], o1[:], AF.Copy, accum_out=mu_parts[:, 0:1]
            )
            nc.scalar.activation(
                res_sb[:, 512:768], o2[:], AF.Copy, accum_out=mu_parts[:, 1:2]
            )
            mu = fin.tile([S, 1], F32, tag="mu", name="mu")
            nc.vector.tensor_reduce(
                mu[:], mu_parts[:], axis=mybir.AxisListType.X, op=ALU.add
            )
            muv = fin.tile([S, 1], F32, tag="muv", name="muv")
            nc.vector.tensor_scalar_mul(muv[:], mu[:], 1.0 / HIDDEN)
            xc = fin.tile([S, HIDDEN], F32, tag="xc", name="xc")
            nc.vector.tensor_scalar_sub(xc[:], res_sb[:], muv[:])
            sq = fin.tile([S, HIDDEN], F32, tag="sq", name="sq")
            varsum = fin.tile([S, 1], F32, tag="vs", name="varsum")
            nc.scalar.activation(sq[:], xc[:], AF.Square, accum_out=varsum[:])
            vareps = fin.tile([S, 1], F32, tag="ve", name="vareps")
            nc.vector.tensor_scalar(
                vareps[:], varsum[:], 1.0 / HIDDEN, LN_EPS, op0=ALU.mult, op1=ALU.add
            )
            sd = fin.tile([S, 1], F32, tag="sd", name="sd")
            nc.scalar.activation(sd[:], vareps[:], AF.Sqrt)
            rstd = fin.tile([S, 1], F32, tag="rstd", name="rstd")
            nc.vector.reciprocal(rstd[:], sd[:])
            t1 = fin.tile([S, HIDDEN], F32, tag="t1", name="t1")
            nc.vector.scalar_tensor_tensor(
                t1[:], xc[:], rstd[:], gamma_rep[:], op0=ALU.mult, op1=ALU.mult
            )
            out_sb = fin.tile([S, HIDDEN], F32, tag="osb", name="out_sb")
            nc.vector.tensor_add(out_sb[:], t1[:], beta_rep[:])
            nc.sync.dma_start(out[:], out_sb[:])


_NC_CACHE = {}


def kernel(hidden_states, mask, Wq, bq, Wv, bv, Wo, bo, gamma, beta):
    if "nc" not in _NC_CACHE:
        _NC_CACHE["nc"] = build_kernel()
    nc = _NC_CACHE["nc"]
    ident = np.eye(128, dtype=np.float32)
    f32 = np.float32

    def cc(a):
        return np.ascontiguousarray(a, dtype=f32)

    in_maps = [
        {
            "hs": cc(hidden_states[b]),
            "mask": cc(mask[b]),
            "wq": cc(Wq),
            "bq": cc(bq),
            "wv": cc(Wv),
            "bv": cc(bv),
            "wo": cc(Wo),
            "bo": cc(bo),
            "gamma": cc(gamma),
            "beta": cc(beta),
            "ident": ident,
        }
        for b in range(N_CORES)
    ]
    res = run_bass_kernel_spmd(nc, in_maps, core_ids=list(range(N_CORES)))
    _NC_CACHE["last_results"] = res
    globals()["_LAST_RESULTS"] = res
    return np.stack([res.results[i]["out"] for i in range(N_CORES)], axis=0)


# revision 15
# speedup vs baseline: 1.4833x; 1.2182x over previous
"""Trainium2 Bass kernel for nn_MultiHeadSelfTokenAttention.

Reference computation (per (b, s) slice, X = hidden[b, s] in [T=128, H=768]):
    q      = X @ Wq + bq                       [T, 12]     (per-token per-head logit)
    scores = q + mask[:, None] * (-10000)
    alpha  = softmax(scores, axis=T)           [T, 12]
    v      = (X @ Wv + bv).reshape(T, 12, 64)
    res    = einsum('th,thd->hd', alpha, v)    [12, 64] -> [768]
    out    = LN(res @ Wo + bo) * gamma + beta  [768]

Key algebraic restructure (makes the kernel memory-bound instead of
compute-bound): the pooled value P = sum_t alpha * V is computed as
    Y[head, h] = sum_t alpha[t, head] * X[t, h]
    P[head, :] = Y[head, :] @ Wv[:, head*64:(head+1)*64] + bv_head
so V ([T, 768] per slice) is never materialized; the X@Wv matmul
(151 MFLOP/slice) collapses to ~2.4 MFLOP/slice.

Sharding: data-parallel across batch; core b handles hidden_states[b]
(32 sents).  Weights replicated.  No collectives.
"""

import os
import sys
from contextlib import ExitStack

import numpy as np

for _p in ("/opt/trn_rl_repo", "/root/.axon_site/_ro/trn_rl_repo"):
    if os.path.isdir(_p) and _p not in sys.path:
        sys.path.insert(0, _p)

import concourse.bacc as bacc
import concourse.bass as bass
import concourse.tile as tile
from concourse import mybir
from concourse.bass_utils import run_bass_kernel_spmd

F32 = mybir.dt.float32
F32R = mybir.dt.float32r
AF = mybir.ActivationFunctionType
ALU = mybir.AluOpType

HIDDEN = 768
HEADS = 12
B, S, T = 8, 32, 128
HC = HIDDEN // 128  # 6 chunks of the hidden dim
LN_EPS = 1e-5
MASK_NEG = -10000.0
N_CORES = 8
BS = 4  # sents per block
NBLK = S // BS

# dtype knob: float32r tiles feed 1-pass (4x faster, ~1.5e-4 rel) PE matmuls;
# float32 gives the exact 2-pass path.
MMD = F32R if os.environ.get("KMM", "f32r") == "f32r" else F32


def _f32(ap):
    # exact-bits view of an MMD tile for non-matmul consumers (PE transpose)
    return ap.bitcast(F32) if MMD != F32 else ap


def build_kernel():
    nc = bacc.Bacc(trn_type="TRN2", target_bir_lowering=False, debug=False)

    hs = nc.dram_tensor("hs", [S, T, HIDDEN], MMD, kind="ExternalInput").ap()
    mask = nc.dram_tensor("mask", [S, T], F32, kind="ExternalInput").ap()
    wq = nc.dram_tensor("wq", [HIDDEN, HEADS], MMD, kind="ExternalInput").ap()
    bq = nc.dram_tensor("bq", [HEADS], F32, kind="ExternalInput").ap()
    wv = nc.dram_tensor("wv", [HIDDEN, HIDDEN], MMD, kind="ExternalInput").ap()
    bv = nc.dram_tensor("bv", [HIDDEN], F32, kind="ExternalInput").ap()
    wo = nc.dram_tensor("wo", [HIDDEN, HIDDEN], MMD, kind="ExternalInput").ap()
    bo = nc.dram_tensor("bo", [HIDDEN], F32, kind="ExternalInput").ap()
    gamma = nc.dram_tensor("gamma", [HIDDEN], F32, kind="ExternalInput").ap()
    beta = nc.dram_tensor("beta", [HIDDEN], F32, kind="ExternalInput").ap()
    ident = nc.dram_tensor("ident", [128, 128], F32, kind="ExternalInput").ap()
    out = nc.dram_tensor("out", [S, HIDDEN], F32, kind="ExternalOutput").ap()

    with tile.TileContext(nc) as tc:
        kernel_body(tc, out, hs, mask, wq, bq, wv, bv, wo, bo, gamma, beta, ident)
    nc.compile()
    return nc


def kernel_body(tc, out, hs, mask, wq, bq, wv, bv, wo, bo, gamma, beta, ident):
    nc = tc.nc
    with ExitStack() as ctx:
        consts = ctx.enter_context(tc.tile_pool(name="consts", bufs=1))
        xp = ctx.enter_context(tc.tile_pool(name="x", bufs=2))
        xtp = ctx.enter_context(tc.tile_pool(name="xt", bufs=2))
        smallp = ctx.enter_context(tc.tile_pool(name="small", bufs=2))
        psctx = ExitStack()
        ps_xt = psctx.enter_context(tc.tile_pool(name="ps_xt", bufs=2, space="PSUM"))
        ps_qt = psctx.enter_context(tc.tile_pool(name="ps_qt", bufs=1, space="PSUM"))
        ps_et = psctx.enter_context(tc.tile_pool(name="ps_et", bufs=1, space="PSUM"))
        ps_yt = psctx.enter_context(tc.tile_pool(name="ps_yt", bufs=1, space="PSUM"))

        # ---------------- constants / weights -------------------------------
        ident_sb = consts.tile([128, 128], F32, tag="ident")
        nc.sync.dma_start(ident_sb[:], ident[:])

        # wq_sb[p, c*12+n] = Wq[c*128+p, n]
        wq_sb = consts.tile([128, HC * HEADS], MMD, tag="wq")
        nc.scalar.dma_start(wq_sb[:], wq.rearrange("(c p) n -> p c n", p=128))

        # extras matmul operands: scores^T += [NEG; bq]^T-style rank-2 update
        extras_w = consts.tile([2, HEADS], F32, tag="exw")
        nc.vector.memset(extras_w[0:1, :], MASK_NEG)
        nc.scalar.dma_start(extras_w[1:2, :], bq[None, :])
        extras_rhs = consts.tile([2, S * T], F32, tag="exr")
        nc.vector.memset(extras_rhs[:], 1.0)  # row 1 stays all-ones
        nc.scalar.dma_start(extras_rhs[0:1, :], mask.rearrange("s t -> (s t)")[None, :])

        ones_col = consts.tile([1, S], F32, tag="ones")
        nc.vector.memset(ones_col[:], 1.0)
        bo_row = consts.tile([1, HIDDEN], F32, tag="bo")
        nc.scalar.dma_start(bo_row[:], bo[None, :])
        g_row = consts.tile([1, HIDDEN], F32, tag="grow")
        nc.scalar.dma_start(g_row[:], gamma[None, :])
        b_row = consts.tile([1, HIDDEN], F32, tag="brow")
        nc.scalar.dma_start(b_row[:], beta[None, :])

        # bv_sb[p, c] = bv[c*128+p]
        bv_sb = consts.tile([128, HC], F32, tag="bv")
        nc.scalar.dma_start(bv_sb[:], bv.rearrange("(c p) -> p c", p=128))

        # big weight loads issued last on the scalar ring (needed only at stage G/C)
        # wv_sb[p, c*768+d] = Wv[c*128+p, d]
        wv_sb = consts.tile([128, HC * HIDDEN], MMD, tag="wv")
        nc.scalar.dma_start(wv_sb[:], wv.rearrange("(c p) n -> p c n", p=128))
        wo_sb = consts.tile([128, HC * HIDDEN], MMD, tag="wo")
        nc.scalar.dma_start(wo_sb[:], wo.rearrange("(c p) n -> p c n", p=128))

        # Y^T accumulator in SBUF: per h-chunk [128, S*HEADS], col = s*12 + head
        yt_sb = [
            consts.tile([128, S * HEADS], MMD, tag=f"yt{c}", name=f"yt{c}")
            for c in range(HC)
        ]

        # ---------------- main loop over sent blocks ------------------------
        for blk in range(NBLK):
            s0 = blk * BS
            # X block: [t, (s', h)]  (contiguous DMA, 1.57 MB)
            x_blk = xp.tile([128, BS * HIDDEN], MMD, tag="xblk", name="x_blk")
            nc.sync.dma_start(x_blk[:], hs[s0 : s0 + BS].rearrange("s t h -> t s h"))

            # X^T block in SBUF: col = s'*768 + hc*128 + j
            xt_blk = xtp.tile([128, BS * HIDDEN], MMD, tag="xtblk", name="xt_blk")
            for sp in range(BS):
                xt_ps = ps_xt.tile([128, HIDDEN], F32, tag="xtps", name="xt_ps")
                for c in range(HC):
                    nc.tensor.transpose(
                        xt_ps[:, c * 128 : (c + 1) * 128],
                        _f32(
                            x_blk[
                                :, sp * HIDDEN + c * 128 : sp * HIDDEN + (c + 1) * 128
                            ]
                        ),
                        ident_sb[:],
                    )
                nc.vector.tensor_copy(
                    xt_blk[:, sp * HIDDEN : (sp + 1) * HIDDEN], xt_ps[:]
                )

            # q^T for the whole block: [12, BS*128]
            qt_ps = ps_qt.tile([HEADS, BS * T], F32, tag="qt", name="qt_ps")
            xt_r = xt_blk.rearrange("p (s c j) -> p c s j", s=BS, j=128)
            for c in range(HC):
                nc.tensor.matmul(
                    qt_ps[:],
                    wq_sb[:, c * HEADS : (c + 1) * HEADS],
                    xt_r[:, c],
                    start=(c == 0),
                    stop=False,
                )
            nc.tensor.matmul(
                qt_ps[:],
                extras_w[:],
                extras_rhs[:, s0 * T : (s0 + BS) * T],
                start=False,
                stop=True,
            )

            # softmax pieces (no max-subtraction: unmasked logits are O(5);
            # masked logits are ~-1e4 and exp underflows to exactly 0,
            # matching the reference's max-subtracted exp)
            et_sb = smallp.tile([HEADS, BS * T], F32, tag="et", name="et_sb")
            zsum = smallp.tile([HEADS, BS], F32, tag="zsum", name="zsum")
            for sp in range(BS):
                nc.scalar.activation(
                    et_sb[:, sp * T : (sp + 1) * T],
                    qt_ps[:, sp * T : (sp + 1) * T],
                    AF.Exp,
                    accum_out=zsum[:, sp : sp + 1],
                )
            zinv = smallp.tile([HEADS, BS], F32, tag="zinv", name="zinv")
            nc.vector.reciprocal(zinv[:], zsum[:])

            # normalize: alpha^T = e^T / Z, then transpose to [t, head] layout
            at_sb = smallp.tile([HEADS, BS * T], F32, tag="at", name="at_sb")
            for sp in range(BS):
                nc.vector.tensor_scalar_mul(
                    at_sb[:, sp * T : (sp + 1) * T],
                    et_sb[:, sp * T : (sp + 1) * T],
                    zinv[:, sp : sp + 1],
                )
            ett_ps = ps_et.tile([128, BS * HEADS], F32, tag="ett", name="ett_ps")
            for sp in range(BS):
                nc.tensor.transpose(
                    ett_ps[:, sp * HEADS : (sp + 1) * HEADS],
                    at_sb[:, sp * T : (sp + 1) * T],
                    ident_sb[0:HEADS, 0:HEADS],
                )
            e_sb = smallp.tile([128, BS * HEADS], MMD, tag="e", name="e_sb")
            nc.vector.tensor_copy(e_sb[:], ett_ps[:])

            yt_a = ps_yt.tile([128, 3 * BS * HEADS], F32, tag="yt_a", name="yt_a")
            yt_b = ps_yt.tile([128, 3 * BS * HEADS], F32, tag="yt_b", name="yt_b")
            for sp in range(BS):
                # Y^T chunks directly: YT[h, head] = sum_t X[t, h] alpha[t, head]
                for c in range(HC):
                    bank = yt_a if c < 3 else yt_b
                    cc = c % 3
                    nc.tensor.matmul(
                        bank[
                            :,
                            cc * BS * HEADS
                            + sp * HEADS : cc * BS * HEADS
                            + (sp + 1) * HEADS,
                        ],
                        x_blk[
                            :, sp * HIDDEN + c * 128 : sp * HIDDEN + (c + 1) * 128
                        ],
                        e_sb[:, sp * HEADS : (sp + 1) * HEADS],
                    )
            for c in range(HC):
                bank = yt_a if c < 3 else yt_b
                cc = c % 3
                eng = nc.vector.tensor_copy if c % 2 == 0 else nc.scalar.copy
                eng(
                    yt_sb[c][:, s0 * HEADS : (s0 + BS) * HEADS],
                    bank[:, cc * BS * HEADS : (cc + 1) * BS * HEADS],
                )

        psctx.close()  # free the main-loop PSUM banks before stage G

        # ---------------- pooled projection through Wv (G-route) ------------
        # G^T[d, (s,head)] = sum_h Wv[h, d] * Y^T[h, (s,head)]; per-head
        # diagonal blocks extracted:  P^T[d, s] = G^T[d, s*12 + head(d)] + bv[d]
        with (
            tc.tile_pool(name="ps_g", bufs=2, space="PSUM") as ps_g,
            tc.tile_pool(name="ps_o", bufs=1, space="PSUM") as ps_o,
            tc.tile_pool(name="fin", bufs=1) as fin,
        ):
            # gamma/beta replicated across the 32 sent-partitions via K=1 matmuls
            gamma_rep = fin.tile([S, HIDDEN], F32, tag="grep", name="gamma_rep")
            beta_rep = fin.tile([S, HIDDEN], F32, tag="brep", name="beta_rep")
            for row, rep in ((g_row, gamma_rep), (b_row, beta_rep)):
                gb1 = ps_g.tile([S, 512], F32, tag="g", name="gb1")
                gb2 = ps_g.tile([S, 256], F32, tag="g", name="gb2")
                nc.tensor.matmul(gb1[:], ones_col[:], row[:, 0:512])
                nc.tensor.matmul(gb2[:], ones_col[:], row[:, 512:768])
                nc.vector.tensor_copy(rep[:, 0:512], gb1[:])
                nc.scalar.copy(rep[:, 512:768], gb2[:])

            pt_sb = fin.tile([128, HC * S], MMD, tag="pt", name="pt_sb")
            for dc in range(HC):
                g_ps = ps_g.tile([128, S * HEADS], F32, tag="g", name="g_ps")
                for c in range(HC):
                    nc.tensor.matmul(
                        g_ps[:],
                        wv_sb[
                            :, c * HIDDEN + dc * 128 : c * HIDDEN + (dc + 1) * 128
                        ],
                        yt_sb[c][:],
                        start=(c == 0),
                        stop=(c == HC - 1),
                    )
                g_r = g_ps.rearrange("p (s n) -> p s n", n=HEADS)
                for half in range(2):
                    head = 2 * dc + half
                    nc.vector.tensor_scalar_add(
                        pt_sb[half * 64 : half * 64 + 64, dc * S : (dc + 1) * S],
                        g_r[half * 64 : half * 64 + 64, :, head],
                        bv_sb[half * 64 : half * 64 + 64, dc : dc + 1],
                    )

            # out = P @ Wo + bo   -> [32, 768]
            o1 = ps_o.tile([S, 512], F32, tag="o1", name="o1")
            o2 = ps_o.tile([S, 256], F32, tag="o2", name="o2")
            for dc in range(HC):
                nc.tensor.matmul(
                    o1[:],
                    pt_sb[:, dc * S : (dc + 1) * S],
                    wo_sb[:, dc * HIDDEN : dc * HIDDEN + 512],
                    start=(dc == 0),
                    stop=False,
                )
                nc.tensor.matmul(
                    o2[:],
                    pt_sb[:, dc * S : (dc + 1) * S],
                    wo_sb[:, dc * HIDDEN + 512 : (dc + 1) * HIDDEN],
                    start=(dc == 0),
                    stop=False,
                )
            nc.tensor.matmul(o1[:], ones_col[:], bo_row[:, 0:512], start=False, stop=True)
            nc.tensor.matmul(
                o2[:], ones_col[:], bo_row[:, 512:768], start=False, stop=True
            )

            # ---------------- layernorm ------------------------------------
            res_sb = fin.tile([S, HIDDEN], F32, tag="res", name="res_sb")
            mu_parts = fin.tile([S, 2], F32, tag="mup", name="mu_parts")
            nc.scalar.activation(
                res_sb[:, 0:512], o1[:], AF.Copy, accum_out=mu_parts[:, 0:1]
            )
            nc.scalar.activation(
                res_sb[:, 512:768], o2[:], AF.Copy, accum_out=mu_parts[:, 1:2]
            )
            mu = fin.tile([S, 1], F32, tag="mu", name="mu")
            nc.vector.tensor_reduce(
                mu[:], mu_parts[:], axis=mybir.AxisListType.X, op=ALU.add
            )
            muv = fin.tile([S, 1], F32, tag="muv", name="muv")
            nc.vector.tensor_scalar_mul(muv[:], mu[:], 1.0 / HIDDEN)
            xc = fin.tile([S, HIDDEN], F32, tag="xc", name="xc")
            nc.vector.tensor_scalar_sub(xc[:], res_sb[:], muv[:])
            sq = fin.tile([S, HIDDEN], F32, tag="sq", name="sq")
            varsum = fin.tile([S, 1], F32, tag="vs", name="varsum")
            nc.scalar.activation(sq[:], xc[:], AF.Square, accum_out=varsum[:])
            vareps = fin.tile([S, 1], F32, tag="ve", name="vareps")
            nc.vector.tensor_scalar(
                vareps[:], varsum[:], 1.0 / HIDDEN, LN_EPS, op0=ALU.mult, op1=ALU.add
            )
            sd = fin.tile([S, 1], F32, tag="sd", name="sd")
            nc.scalar.activation(sd[:], vareps[:], AF.Sqrt)
            rstd = fin.tile([S, 1], F32, tag="rstd", name="rstd")
            nc.vector.reciprocal(rstd[:], sd[:])
            t1 = fin.tile([S, HIDDEN], F32, tag="t1", name="t1")
            nc.vector.scalar_tensor_tensor(
                t1[:], xc[:], rstd[:], gamma_rep[:], op0=ALU.mult, op1=ALU.mult
            )
            out_sb = fin.tile([S, HIDDEN], F32, tag="osb", name="out_sb")
            nc.vector.tensor_add(out_sb[:], t1[:], beta_rep[:])
            nc.sync.dma_start(out[:], out_sb[:])


_NC_CACHE = {}


def kernel(hidden_states, mask, Wq, bq, Wv, bv, Wo, bo, gamma, beta):
    if "nc" not in _NC_CACHE:
        _NC_CACHE["nc"] = build_kernel()
    nc = _NC_CACHE["nc"]
    ident = np.eye(128, dtype=np.float32)
    f32 = np.float32

    def cc(a):
        return np.ascontiguousarray(a, dtype=f32)

    in_maps = [
        {
            "hs": cc(hidden_states[b]),
            "mask": cc(mask[b]),
            "wq": cc(Wq),
            "bq": cc(bq),
            "wv": cc(Wv),
            "bv": cc(bv),
            "wo": cc(Wo),
            "bo": cc(bo),
            "gamma": cc(gamma),
            "beta": cc(beta),
            "ident": ident,
        }
        for b in range(N_CORES)
    ]
    res = run_bass_kernel_spmd(nc, in_maps, core_ids=list(range(N_CORES)))
    _NC_CACHE["last_results"] = res
    globals()["_LAST_RESULTS"] = res
    return np.stack([res.results[i]["out"] for i in range(N_CORES)], axis=0)
